# revision 30
# baseline (speedup 1.0000x reference)
"""Trainium2 Bass kernel for nn_AdaptiveAttention (sparse cluster attention).

Sharding: 8 cores = 2 batches x 4 query-quarters. Each core computes all 4
heads for its 1024 query positions.

Host-side prep (pure layout work): per batch, sort the pixel axis by
cluster label; per core, rotate the key axis so the core's own labels'
segments sit at the front — every same-label (query, key) pair then falls
in a fixed leading band of key tiles, so the intra-cluster Q@K^T scores
and the predicated blend only run on `band` of the 32 key tiles.
pixel_coefficients is pre-transposed to [key, query] so the device streams
it natively; the large inputs ship as bf16. The 1/sqrt(HD) score scale is
folded into K^T and the cluster centers at projection time.

Device loop: heads processed in pairs with [128, 512]-chunk PSUM tiles so
the TensorEngine sees dense back-to-back matmul work (HAM stays warm) with
pipeline depth 4 between PE and the DVE/ACT blend/exp stages.
"""
import sys

sys.path.insert(0, "/opt/trn_rl_repo")

import ml_dtypes
import numpy as np

import concourse.bass as bass
import concourse.tile as tile
from concourse import bacc, mybir
from concourse.bass_utils import run_bass_kernel_spmd

F32 = mybir.dt.float32
BF16 = mybir.dt.bfloat16
U8 = mybir.dt.uint8
AF = mybir.ActivationFunctionType
ALU = mybir.AluOpType
NPBF16 = ml_dtypes.bfloat16

B, C, H, W = 2, 256, 64, 64
NH, HD, K = 4, 64, 16
N = H * W            # 4096
NQ = N // 4          # 1024 queries per core
SCALE = float(HD) ** 0.5
ISCALE = 1.0 / SCALE
NT = N // 128        # 32 key tiles

TRACE = False
TRACE_KW = {}

LDW_OPT = False
import concourse.bass_utils as _bu
if not hasattr(_bu, "_orig_run_command"):
    _bu._orig_run_command = _bu.run_command

    def _patched_run_command(cmd, *a, **kw):
        if LDW_OPT and isinstance(cmd, list):
            cmd = ["--enable-ldw-opt=true" if c == "--enable-ldw-opt=false" else c
                   for c in cmd]
        return _bu._orig_run_command(cmd, *a, **kw)

    _bu.run_command = _patched_run_command


def build(band):
    nc = bacc.Bacc(None, target_bir_lowering=False)

    d_xq = nc.declare_dram_parameter("xq", [C, NQ], BF16, isOutput=False)
    d_xvq = nc.declare_dram_parameter("xvq", [C, NQ], BF16, isOutput=False)
    d_xk = nc.declare_dram_parameter("xk", [C, N], BF16, isOutput=False)
    d_xv = nc.declare_dram_parameter("xv", [C, N], BF16, isOutput=False)
    d_pct = nc.declare_dram_parameter("pct", [N, NQ], BF16, isOutput=False)
    d_oh = nc.declare_dram_parameter("oh", [N, K], BF16, isOutput=False)
    d_ohT = nc.declare_dram_parameter("ohT", [K, N], BF16, isOutput=False)
    d_smask = nc.declare_dram_parameter("smask", [band * 128, NQ], U8, isOutput=False)
    d_recip = nc.declare_dram_parameter("recip", [K, 1], F32, isOutput=False)
    d_wq = nc.declare_dram_parameter("wq", [C, C], BF16, isOutput=False)
    d_wk = nc.declare_dram_parameter("wk", [C, C], BF16, isOutput=False)
    d_wv = nc.declare_dram_parameter("wv", [C, C], BF16, isOutput=False)
    d_bq = nc.declare_dram_parameter("bq", [C, 1], F32, isOutput=False)
    d_bk8 = nc.declare_dram_parameter("bk8", [C, 1], F32, isOutput=False)
    d_bv = nc.declare_dram_parameter("bv", [C, 1], F32, isOutput=False)
    d_bkB = nc.declare_dram_parameter("bkB", [128, C], F32, isOutput=False)
    d_bvB = nc.declare_dram_parameter("bvB", [128, C], F32, isOutput=False)
    d_w11 = nc.declare_dram_parameter("w11", [C, 2 * C], BF16, isOutput=False)
    d_b11 = nc.declare_dram_parameter("b11", [2 * C, 1], F32, isOutput=False)
    d_w12 = nc.declare_dram_parameter("w12", [2 * C, C], BF16, isOutput=False)
    d_b12 = nc.declare_dram_parameter("b12", [C, 1], F32, isOutput=False)
    d_w21 = nc.declare_dram_parameter("w21", [C, 2 * C], BF16, isOutput=False)
    d_b21 = nc.declare_dram_parameter("b21", [2 * C, 1], F32, isOutput=False)
    d_w22 = nc.declare_dram_parameter("w22", [2 * C, C], BF16, isOutput=False)
    d_b22 = nc.declare_dram_parameter("b22", [C, 1], F32, isOutput=False)
    d_out = nc.declare_dram_parameter("out", [C, NQ], F32, isOutput=True)

    with tile.TileContext(nc) as tc:
        with (
            tc.tile_pool(name="persist", bufs=1) as pp,
            tc.tile_pool(name="pct1", bufs=1) as pctp1,
        ):
            # ---- persistent SBUF ----
            KT = [pp.tile([128, N], BF16, name=f"KT{i}") for i in range(2)]
            QT = [pp.tile([128, NQ], BF16, name=f"QT{i}") for i in range(2)]
            CT = [pp.tile([128, N], BF16, name=f"CT{i}") for i in range(2)]
            Vs = pp.tile([128, NT * NH * 65], BF16, name="Vs")
            vT = [pp.tile([128, NQ], F32, name=f"vT{i}") for i in range(2)]
            ohT_bf = pp.tile([K, N], BF16, name="ohT_bf")
            QTz = [pp.tile([128, NQ], BF16, name=f"QTz{i}") for i in range(4)]
            rrs2 = [pp.tile([128, NQ], F32, name=f"rrs{i}") for i in range(2)]
            lnr_p = pp.tile([1, NQ], F32, name="lnr_p")
            outT = [pp.tile([128, NQ], F32, name=f"outT{i}") for i in range(2)]
            pct_bf1 = pctp1.tile([128, 16 * NQ], BF16, name="pct_bf1")

            w11_bf = [pp.tile([128, 2 * C], BF16, name=f"w11_{i}") for i in range(2)]
            w12_bf = [pp.tile([128, C], BF16, name=f"w12_{i}") for i in range(4)]
            w21_bf = [pp.tile([128, 2 * C], BF16, name=f"w21_{i}") for i in range(2)]
            w22_bf = [pp.tile([128, C], BF16, name=f"w22_{i}") for i in range(4)]
            b11_c = [pp.tile([128, 1], F32, name=f"b11_{i}") for i in range(4)]
            b12_c = [pp.tile([128, 1], F32, name=f"b12_{i}") for i in range(2)]
            b21_c = [pp.tile([128, 1], F32, name=f"b21_{i}") for i in range(4)]
            b22_c = [pp.tile([128, 1], F32, name=f"b22_{i}") for i in range(2)]
            ones64 = pp.tile([1, 64], F32, name="ones64")
            ones64_z = pp.tile([128, 64], F32, name="ones64_z")
            xvq_bf = [pp.tile([128, NQ], BF16, name=f"xvq_{i}") for i in range(2)]
            wv_bf = [pp.tile([128, C], BF16, name=f"wv_{i}") for i in range(2)]
            bv_c = [pp.tile([128, 1], F32, name=f"bv_{i}") for i in range(2)]


            with (
                tc.tile_pool(name="stage", bufs=1) as sp,
                tc.tile_pool(name="psA", bufs=2, space=bass.MemorySpace.PSUM) as psA,
                tc.tile_pool(name="psB", bufs=4, space=bass.MemorySpace.PSUM) as psB,
                tc.tile_pool(name="psC", bufs=1, space=bass.MemorySpace.PSUM) as psC,
            ):
                # ---- staging loads (all HWDGE, dtypes match DRAM) ----
                xq_bf = [sp.tile([128, NQ], BF16, name=f"xq_{i}") for i in range(2)]
                xk_bf = [sp.tile([128, N], BF16, name=f"xk_{i}") for i in range(2)]
                xv_bf = [sp.tile([128, N], BF16, name=f"xv_{i}") for i in range(2)]
                wq_bf = [sp.tile([128, C], BF16, name=f"wq_{i}") for i in range(2)]
                wk_bf = [sp.tile([128, C], BF16, name=f"wk_{i}") for i in range(2)]
                bq_c = [sp.tile([128, 1], F32, name=f"bq_{i}") for i in range(2)]
                bk8_c = [sp.tile([128, 1], F32, name=f"bk8_{i}") for i in range(2)]
                bkB_s = sp.tile([128, C], F32, name="bkB_s")
                bvB_s = sp.tile([128, C], F32, name="bvB_s")
                K_rm = sp.tile([128, NT * C], BF16, name="K_rm")
                oh_bf = sp.tile([128, NT * K], BF16, name="oh_bf")
                recip_s = sp.tile([K, 1], F32, name="recip_s")
                cen_bf = sp.tile([K, C], BF16, name="cen_bf")

                nc.sync.dma_start(bkB_s[:], d_bkB[:])
                nc.sync.dma_start(recip_s[:], d_recip[:])
                nc.sync.dma_start(
                    oh_bf[:].rearrange("p (i k) -> p i k", k=K),
                    d_oh[:].rearrange("(i p) k -> p i k", p=128),
                )
                for t in range(2):
                    r = slice(t * 128, (t + 1) * 128)
                    nc.sync.dma_start(wk_bf[t][:], d_wk[r, :])
                    nc.sync.dma_start(xk_bf[t][:], d_xk[r, :])
                for t in range(2):
                    r = slice(t * 128, (t + 1) * 128)
                    nc.sync.dma_start(xq_bf[t][:], d_xq[r, :])
                    nc.sync.dma_start(xvq_bf[t][:], d_xvq[r, :])
                    nc.sync.dma_start(xv_bf[t][:], d_xv[r, :])
                    nc.sync.dma_start(wq_bf[t][:], d_wq[r, :])
                    nc.sync.dma_start(wv_bf[t][:], d_wv[r, :])
                    nc.sync.dma_start(bq_c[t][:], d_bq[r, :])
                    nc.sync.dma_start(bk8_c[t][:], d_bk8[r, :])
                    nc.sync.dma_start(bv_c[t][:], d_bv[r, :])
                    nc.sync.dma_start(w11_bf[t][:], d_w11[r, :])
                    nc.sync.dma_start(w21_bf[t][:], d_w21[r, :])
                    nc.sync.dma_start(b12_c[t][:], d_b12[r, :])
                    nc.sync.dma_start(b22_c[t][:], d_b22[r, :])
                for t in range(4):
                    r = slice(t * 128, (t + 1) * 128)
                    nc.sync.dma_start(w12_bf[t][:], d_w12[r, :])
                    nc.sync.dma_start(w22_bf[t][:], d_w22[r, :])
                    nc.sync.dma_start(b11_c[t][:], d_b11[r, :])
                    nc.sync.dma_start(b21_c[t][:], d_b21[r, :])
                nc.sync.dma_start(bvB_s[:], d_bvB[:])
                nc.sync.dma_start(ohT_bf[:], d_ohT[:])
                nc.gpsimd.memset(ones64[:], 1.0)
                nc.gpsimd.memset(ones64_z[:], 1.0)

                # pct first half, behind a token dep on xk so the staging
                # loads get the DMA bandwidth first
                pct_gate = sp.tile([1, 1], BF16, name="pct_gate")
                nc.gpsimd.tensor_copy(pct_gate[:], xk_bf[1][0:1, 0:1])
                for cidx in range(4):
                    psrc = d_pct[cidx * 512:(cidx + 1) * 512, :].rearrange(
                        "(j p) c -> p j c", p=128)
                    pdst = pct_bf1[:, cidx * 4 * NQ:(cidx + 1) * 4 * NQ].rearrange(
                        "p (j c) -> p j c", c=NQ)
                    nc.gpsimd.dma_start(pdst, psrc)

                # ---- projections ----
                # K row-major first (centers gate the attention loop), Q^T
                for i in range(NT):
                    ps = psB.tile([128, C], F32, name="psv")
                    for c in range(2):
                        nc.tensor.matmul(
                            ps[:], xk_bf[c][:, i * 128:(i + 1) * 128],
                            wk_bf[c][:], start=(c == 0), stop=(c == 1))
                    nc.vector.scalar_tensor_tensor(
                        K_rm[:, i * C:(i + 1) * C], ps[:], 1.0, bkB_s[:],
                        ALU.mult, ALU.add)
                for co in range(2):
                    for ch in range(NQ // 512):
                        ps = psA.tile([128, 512], F32, name="ps")
                        for c in range(2):
                            nc.tensor.matmul(
                                ps[:], wq_bf[c][:, co * 128:(co + 1) * 128],
                                xq_bf[c][:, ch * 512:(ch + 1) * 512],
                                start=(c == 0), stop=(c == 1))
                        nc.vector.tensor_scalar(
                            QT[co][:, ch * 512:(ch + 1) * 512], ps[:],
                            bq_c[co][:], None, op0=ALU.add)

                # per-head zero-padded Q^T: full-128-contraction score MMs
                # keep the PE activity monitor warm (half-height MMs throttle)
                for h in range(NH):
                    z = QTz[h]
                    nc.vector.memset(z[:], 0.0)
                    pr = slice((h % 2) * 64, (h % 2) * 64 + 64)
                    nc.scalar.activation(z[pr, :], QT[h // 2][pr, :], AF.Copy)

                # centers (pre-scaled by 1/sqrt(HD) via recip input)
                psc = psC.tile([K, C], F32, name="psc")
                for i in range(NT):
                    nc.tensor.matmul(
                        psc[:], oh_bf[:, i * K:(i + 1) * K],
                        K_rm[:, i * C:(i + 1) * C],
                        start=(i == 0), stop=(i == NT - 1))
                nc.vector.tensor_scalar(
                    cen_bf[:], psc[:], recip_s[:], None, op0=ALU.mult)

                # gathered centers^T: CT[d, m] = centers^T[:, lab[m]]
                for h in range(NH):
                    dst = CT[h // 2]
                    pr = slice((h % 2) * 64, (h % 2) * 64 + 64)
                    for ch in range(N // 512):
                        ps = psC.tile([64, 512], F32, name="psct")
                        nc.tensor.matmul(
                            ps[:], cen_bf[:, h * 64:(h + 1) * 64],
                            ohT_bf[:, ch * 512:(ch + 1) * 512])
                        nc.scalar.activation(
                            dst[pr, ch * 512:(ch + 1) * 512], ps[:], AF.Copy)

                # K^T (pre-scaled), V row-major, v^T
                for co in range(2):
                    for ch in range(N // 512):
                        ps = psA.tile([128, 512], F32, name="ps")
                        for c in range(2):
                            nc.tensor.matmul(
                                ps[:], wk_bf[c][:, co * 128:(co + 1) * 128],
                                xk_bf[c][:, ch * 512:(ch + 1) * 512],
                                start=(c == 0), stop=(c == 1))
                        nc.vector.tensor_scalar(
                            KT[co][:, ch * 512:(ch + 1) * 512], ps[:],
                            ISCALE, bk8_c[co][:], op0=ALU.mult, op1=ALU.add)
                for i in range(NT):
                    ps = psB.tile([128, C], F32, name="psv")
                    for c in range(2):
                        nc.tensor.matmul(
                            ps[:], xv_bf[c][:, i * 128:(i + 1) * 128],
                            wv_bf[c][:], start=(c == 0), stop=(c == 1))
                    nc.vector.scalar_tensor_tensor(
                        Vs[:, i * NH * 65:(i + 1) * NH * 65].rearrange(
                            "p (h s) -> p h s", s=65)[:, :, 0:64],
                        ps[:].rearrange("p (h s) -> p h s", s=64),
                        1.0,
                        bvB_s[:].rearrange("p (h s) -> p h s", s=64),
                        ALU.mult, ALU.add)
                nc.gpsimd.memset(
                    Vs[:].rearrange("p (t s) -> p t s", s=65)[:, :, 64:65], 1.0)


            # second half of pct (reuses the space the staging pool freed)
            pctp2_cm = tc.tile_pool(name="pct2", bufs=1)
            pctp2 = pctp2_cm.__enter__()
            pct_bf2 = pctp2.tile([128, 16 * NQ], BF16, name="pct_bf2")
            for cidx in range(4):
                psrc = d_pct[2048 + cidx * 512:2048 + (cidx + 1) * 512, :].rearrange(
                    "(j p) c -> p j c", p=128)
                pdst = pct_bf2[:, cidx * 4 * NQ:(cidx + 1) * 4 * NQ].rearrange(
                    "p (j c) -> p j c", c=NQ)
                nc.gpsimd.dma_start(pdst, psrc)

            # interleave band (blend path) and plain key tiles for pipelining
            order = []
            bi, ni = 0, band
            for s in range(NT):
                if (s % 2 == 0 or bi >= band) and ni < NT:
                    order.append(ni)
                    ni += 1
                else:
                    order.append(bi)
                    bi += 1

            # ---- attention: lagged-AV emission keeps PE dense ----
            LAG = 5
            with (
                tc.tile_pool(name="sc", bufs=4) as scp,
                tc.tile_pool(name="pb", bufs=LAG + 3) as pbp,
                tc.tile_pool(name="sm", bufs=2) as smp,
                tc.tile_pool(name="psp", bufs=3, space=bass.MemorySpace.PSUM) as psp,
                tc.tile_pool(name="pso", bufs=1, space=bass.MemorySpace.PSUM) as pso,
            ):
                for h in range(NH):
                    src_t = h // 2
                    pr = slice((h % 2) * 64, (h % 2) * 64 + 64)
                    out_ps = pso.tile([65, NQ], F32, name="out_ps")
                    pbufs = {}

                    def emit_av(oidx, i):
                        nonlocal pbufs
                        vsl = Vs[:, (i * NH + h) * 65:(i * NH + h + 1) * 65]
                        p_bf = pbufs.pop(i)
                        for ch in range(NQ // 512):
                            cs = slice(ch * 512, (ch + 1) * 512)
                            nc.tensor.matmul(
                                out_ps[:, cs], vsl, p_bf[:, cs],
                                start=(oidx == 0), stop=(oidx == NT - 1))

                    for oidx, i in enumerate(order):
                        ms = slice(i * 128, (i + 1) * 128)
                        pcsrc = (pct_bf1[:, i * NQ:(i + 1) * NQ] if i < 16
                                 else pct_bf2[:, (i - 16) * NQ:(i - 15) * NQ])
                        ps_b = psp.tile([128, NQ], F32, name="ps")
                        for ch in range(NQ // 512):
                            cs = slice(ch * 512, (ch + 1) * 512)
                            nc.tensor.matmul(ps_b[:, cs], CT[src_t][:, ms],
                                             QTz[h][:, cs])
                        t_bf = scp.tile([128, NQ], BF16, name="t")
                        if i >= band and oidx % 3 == 2:
                            # shift this tile's PSUM read to the ACT engine;
                            # the multiply then runs all-SBUF at 2x on DVE
                            b_s = scp.tile([128, NQ], BF16, name="b_s", bufs=1)
                            nc.scalar.activation(b_s[:], ps_b[:], AF.Copy)
                            nc.vector.tensor_tensor(
                                t_bf[:], pcsrc, b_s[:], ALU.mult)
                        else:
                            nc.vector.tensor_tensor(
                                t_bf[:], pcsrc, ps_b[:], ALU.mult)
                        if i < band:
                            ps_a = psp.tile([128, NQ], F32, name="ps")
                            for ch in range(NQ // 512):
                                cs = slice(ch * 512, (ch + 1) * 512)
                                nc.tensor.matmul(ps_a[:, cs], KT[src_t][:, ms],
                                                 QTz[h][:, cs])
                            sm = smp.tile([128, NQ], U8, name="sm")
                            nc.sync.dma_start(sm[:], d_smask[ms, :])
                            nc.vector.copy_predicated(t_bf[:], sm[:], ps_a[:])
                        p_bf = pbp.tile([128, NQ], BF16, name="p_bf")
                        nc.scalar.activation(p_bf[:], t_bf[:], AF.Exp)
                        pbufs[i] = p_bf
                        if oidx >= LAG:
                            emit_av(oidx - LAG, order[oidx - LAG])
                    for oidx in range(NT - LAG, NT):
                        emit_av(oidx, order[oidx])

                    # stash raw sums; normalization deferred to MLP input.
                    # 1/rowsum via exp(-ln x) on the idle-ish ACT engine
                    rsl = rrs2[h // 2][(h % 2) * 64:(h % 2) * 64 + 1, :]
                    nc.scalar.activation(lnr_p[:], out_ps[64:65, :], AF.Ln)
                    nc.scalar.activation(outT[src_t][pr, :], out_ps[0:64, :],
                                         AF.Copy)
                    nc.scalar.activation(rsl, lnr_p[:], AF.Exp, scale=-1.0)

            pctp2_cm.__exit__(None, None, None)

            # ---- MLPs ----
            with (
                tc.tile_pool(name="mlp", bufs=1) as mp,
                tc.tile_pool(name="psm", bufs=4, space=bass.MemorySpace.PSUM) as psm,
            ):
                out_bf = [mp.tile([128, NQ], BF16, name=f"outbf{i}") for i in range(2)]
                h1_bf = [mp.tile([128, NQ], BF16, name=f"h1_{i}") for i in range(4)]
                rs1 = [mp.tile([128, NQ], F32, name=f"rs1_{i}") for i in range(2)]
                rs1_bf = [mp.tile([128, NQ], BF16, name=f"rs1bf{i}") for i in range(2)]
                fin = [mp.tile([128, NQ], F32, name=f"fin{i}") for i in range(2)]

                for co in range(2):
                    for ch in range(NQ // 512):
                        ps = psm.tile([128, 512], F32, name="psmt")
                        for c in range(2):
                            nc.tensor.matmul(
                                ps[:], wv_bf[c][:, co * 128:(co + 1) * 128],
                                xvq_bf[c][:, ch * 512:(ch + 1) * 512],
                                start=(c == 0), stop=(c == 1))
                        nc.vector.tensor_scalar(
                            vT[co][:, ch * 512:(ch + 1) * 512], ps[:],
                            bv_c[co][:], None, op0=ALU.add)
                for t in range(2):
                    rbp = psm.tile([128, 512], F32, name="psmt")
                    rbp2 = psm.tile([128, 512], F32, name="psmt")
                    for hh in range(2):
                        h = 2 * t + hh
                        rs = slice(hh * 64, hh * 64 + 64)
                        rsrc = rrs2[h // 2]
                        rrow = slice((h % 2) * 64, (h % 2) * 64 + 1)
                        nc.tensor.matmul(rbp[rs, :], ones64_z[rrow, :],
                                         rsrc[rrow, 0:512])
                        nc.tensor.matmul(rbp2[rs, :], ones64_z[rrow, :],
                                         rsrc[rrow, 512:1024])
                    nc.vector.tensor_tensor(
                        out_bf[t][:, 0:512], outT[t][:, 0:512], rbp[:], ALU.mult)
                    nc.vector.tensor_tensor(
                        out_bf[t][:, 512:1024], outT[t][:, 512:1024], rbp2[:],
                        ALU.mult)
                for t in range(4):
                    ts = slice(t * 128, (t + 1) * 128)
                    for ch in range(NQ // 512):
                        cs = slice(ch * 512, (ch + 1) * 512)
                        ps = psm.tile([128, 512], F32, name="psmt")
                        for c in range(2):
                            nc.tensor.matmul(
                                ps[:], w11_bf[c][:, ts], out_bf[c][:, cs],
                                start=(c == 0), stop=(c == 1))
                        nc.scalar.activation(
                            h1_bf[t][:, cs], ps[:], AF.Lrelu,
                            bias=b11_c[t][:], alpha=0.01)
                for co in range(2):
                    cos = slice(co * 128, (co + 1) * 128)
                    for ch in range(NQ // 512):
                        cs = slice(ch * 512, (ch + 1) * 512)
                        ps = psm.tile([128, 512], F32, name="psmt")
                        for t in range(4):
                            nc.tensor.matmul(
                                ps[:], w12_bf[t][:, cos], h1_bf[t][:, cs],
                                start=(t == 0), stop=(t == 3))
                        nc.vector.scalar_tensor_tensor(
                            rs1[co][:, cs], ps[:], b12_c[co][:], vT[co][:, cs],
                            ALU.add, ALU.add)
                    nc.scalar.activation(rs1_bf[co][:], rs1[co][:], AF.Copy)
                for t in range(4):
                    ts = slice(t * 128, (t + 1) * 128)
                    for ch in range(NQ // 512):
                        cs = slice(ch * 512, (ch + 1) * 512)
                        ps = psm.tile([128, 512], F32, name="psmt")
                        for c in range(2):
                            nc.tensor.matmul(
                                ps[:], w21_bf[c][:, ts], rs1_bf[c][:, cs],
                                start=(c == 0), stop=(c == 1))
                        nc.scalar.activation(
                            h1_bf[t][:, cs], ps[:], AF.Lrelu,
                            bias=b21_c[t][:], alpha=0.01)
                for co in range(2):
                    cos = slice(co * 128, (co + 1) * 128)
                    for ch in range(NQ // 512):
                        cs = slice(ch * 512, (ch + 1) * 512)
                        ps = psm.tile([128, 512], F32, name="psmt")
                        for t in range(4):
                            nc.tensor.matmul(
                                ps[:], w22_bf[t][:, cos], h1_bf[t][:, cs],
                                start=(t == 0), stop=(t == 3))
                        nc.vector.scalar_tensor_tensor(
                            fin[co][:, cs], ps[:], b22_c[co][:],
                            rs1[co][:, cs], ALU.add, ALU.add)
                for co in range(2):
                    nc.sync.dma_start(
                        d_out[co * 128:(co + 1) * 128, :], fin[co][:])

    nc.compile()
    return nc


_NC_CACHE = {}


def _get_nc(band):
    if band not in _NC_CACHE:
        _NC_CACHE[band] = build(band)
    return _NC_CACHE[band]


def kernel(q_img, k_img_map, v_img_map, labels, pixel_coefficients,
           Wq, bq, Wk, bk, Wv, bv, W11, b11, W12, b12, W21, b21, W22, b22):
    q_img = np.asarray(q_img, np.float32)
    k_img_map = np.asarray(k_img_map, np.float32)
    v_img_map = np.asarray(v_img_map, np.float32)
    labels = np.asarray(labels)
    pc = np.asarray(pixel_coefficients, np.float32)
    f = lambda a: np.ascontiguousarray(np.asarray(a, np.float32))
    bf = lambda a: np.ascontiguousarray(np.asarray(a, np.float32).astype(NPBF16))
    col = lambda a: f(a).reshape(-1, 1)

    shared = {
        "wq": bf(Wq), "wk": bf(Wk), "wv": bf(Wv),
        "bq": col(bq), "bk8": col(bk) * ISCALE, "bv": col(bv),
        "bkB": np.tile(f(bk)[None, :], (128, 1)),
        "bvB": np.tile(f(bv)[None, :], (128, 1)),
        "w11": bf(W11), "b11": col(b11), "w12": bf(W12), "b12": col(b12),
        "w21": bf(W21), "b21": col(b21), "w22": bf(W22), "b22": col(b22),
    }

    # per-batch sorted layouts; per-core rotation puts each core's own
    # label segments at the front of the key axis
    percore = []
    perms = []
    band = 0
    for b in range(B):
        lab = labels[b].reshape(N).astype(np.int64)
        perm = np.argsort(lab, kind="stable")
        perms.append(perm)
        labs = lab[perm]
        cnt = np.bincount(labs, minlength=K)
        seg_start = np.concatenate([[0], np.cumsum(cnt)])
        recip = (ISCALE / (cnt + 1e-6)).astype(np.float32).reshape(K, 1)
        xq = np.ascontiguousarray(q_img[b].reshape(C, N)[:, perm]).astype(NPBF16)
        xk = np.ascontiguousarray(k_img_map[b].reshape(C, N)[:, perm]).astype(NPBF16)
        xv = np.ascontiguousarray(v_img_map[b].reshape(C, N)[:, perm]).astype(NPBF16)
        pcT = np.ascontiguousarray(pc[b][perm][:, perm].T).astype(NPBF16)
        for q in range(4):
            cs = slice(q * NQ, (q + 1) * NQ)
            labq = labs[cs]
            lab_a, lab_b = int(labq[0]), int(labq[-1])
            rot = int(seg_start[lab_a])
            span = int(seg_start[lab_b + 1]) - rot
            band = max(band, (span + 127) // 128)
            percore.append((rot, labq, labs, recip, xq, xk, xv, pcT, cs))

    in_maps = []
    for (rot, labq, labs, recip, xq, xk, xv, pcT, cs) in percore:
        ro = (np.arange(N) + rot) % N
        labs_r = labs[ro]
        oh_r = (labs_r[:, None] == np.arange(K)[None, :]).astype(NPBF16)
        smask = (labs_r[:band * 128, None] == labq[None, :]).astype(np.uint8)
        assert not np.isin(labs_r[band * 128:], np.unique(labq)).any()
        m = dict(shared)
        m.update({
            "xq": np.ascontiguousarray(xq[:, cs]),
            "xvq": np.ascontiguousarray(xv[:, cs]),
            "xk": np.ascontiguousarray(xk[:, ro]),
            "xv": np.ascontiguousarray(xv[:, ro]),
            "pct": np.ascontiguousarray(pcT[ro][:, cs]),
            "oh": oh_r,
            "ohT": np.ascontiguousarray(oh_r.T),
            "smask": np.ascontiguousarray(smask),
            "recip": recip,
        })
        in_maps.append(m)

    nc = _get_nc(band)
    res = run_bass_kernel_spmd(nc, in_maps, list(range(8)),
                               trace=TRACE, trace_kwargs=TRACE_KW)
    if TRACE:
        kernel.last_exec_time_ns = res.exec_time_ns
        kernel.last_result = res

    out = np.empty((B, C, H, W), np.float32)
    for b in range(B):
        full_sorted = np.concatenate(
            [res.results[b * 4 + q]["out"] for q in range(4)], axis=1)
        unsorted = np.empty((C, N), np.float32)
        unsorted[:, perms[b]] = full_sorted
        out[b] = unsorted.reshape(C, H, W)
    return out
